# revision 7
# baseline (speedup 1.0000x reference)
"""GroupedQueryAttention (B=2, N=2048, D=2048, H=16, HKV=4, HD=128) on 8 trn2 cores.

Sharding: core c handles (batch b = c//4, kv-head g = c%4): 4 q-heads + 1 kv head.
RoPE (with the reference's sin==cos quirk) is folded into Wq/Wk host-side:
  q_rot = cos ⊙ (M q) with constant M = [[I,-I],[I,I]] acting on head-dim halves,
so on-device RoPE is just an elementwise multiply by a precomputed cos table.
The softmax scale is folded into Wq. All matmuls run in bf16 with fp32 PSUM
accumulation. Attention uses a transpose-free layout chain:
  qT,kT [hd,n]; ST = kT.T @ qT [m,n]; exp on ScalarE; OT += v.T@ST (v as [m,hd]).
The scores loop is software-pipelined one tile ahead so the PE never stalls on
the ScalarE exp. Softmax denominators: DVE pairwise-add tree over the 16 exp
tiles, then a single all-ones stationary matmul per (head, n-chunk); normalize
with DVE fast-reciprocal. Per-head chunked AllGathers (bf16) overlap with
attention of later heads (head 3's gather is split 3+1 n-chunks to shrink the
tail); the output projection accumulates per gathered slab into an SBUF fp32
accumulator in transposed [d, n] layout (Wo chunks stationary), and the host
transposes each core's [512, 2048] slice back.
Host gathers: out[b][:, g*512:(g+1)*512] = core (b,g) output transposed.
"""

import sys
import types

import numpy as np

B, N, D = 2, 2048, 2048
H, HKV, HD = 16, 4, 128
G = H // HKV  # q heads per kv head = 4
N_CORES = 8
ROPE_BASE = 10000.0
DSLICE = D // G  # 512 output columns per core
JL = G * HD  # 512 local attention-output rows per core


def _install_axon_ntff_hook():
    """This container's antenv lacks axon_hooks; inject it so trace=True works."""
    if "antenv.axon_hooks" in sys.modules:
        return
    try:
        from trn_agent_boot.trn_boot import _ntff_profile_via_ctypes

        hook = _ntff_profile_via_ctypes("/opt/axon/libaxon_pjrt.so")
    except Exception:
        hook = None
    mod = types.ModuleType("antenv.axon_hooks")
    mod.get_axon_ntff_profile_hook = lambda: hook
    mod.set_axon_ntff_profile_hook = lambda h: None
    sys.modules["antenv.axon_hooks"] = mod


def _fold_rope(w: np.ndarray, n_heads: int) -> np.ndarray:
    """Return W' with the (sin==cos) RoPE mixing folded in: x@W' = M(x@W) per head."""
    wf = w.reshape(D, n_heads, HD)
    lo, hi = wf[..., : HD // 2], wf[..., HD // 2 :]
    return np.concatenate([lo - hi, hi + lo], axis=-1).reshape(D, n_heads * HD)


def _cos_table() -> np.ndarray:
    inv_freq = 1.0 / (ROPE_BASE ** (np.arange(0, HD, 2, dtype=np.float64) / HD))
    freqs = np.arange(N, dtype=np.float64)[:, None] * inv_freq[None, :]  # [N, 64]
    emb = np.concatenate([freqs, freqs], axis=-1)  # [N, 128]
    return np.cos(emb).T.astype(np.float32).copy()  # [128, N]


_NC_CACHE: dict = {}


def _build_nc():
    if "nc" in _NC_CACHE:
        return _NC_CACHE["nc"]

    import concourse.bacc as bacc
    import concourse.mybir as mybir
    import concourse.tile as tile
    from concourse.bass import ts
    from concourse.masks import make_identity

    f32 = mybir.dt.float32
    bf16 = mybir.dt.bfloat16
    AFT = mybir.ActivationFunctionType
    KD = D // 128  # 16 contraction chunks
    NT = N // 128  # 16 m tiles of 128
    NC512 = N // 512  # 4 chunks of 512
    DC = DSLICE // 128  # 4 output-column chunks of 128

    nc = bacc.Bacc(target_bir_lowering=False, debug=False, num_devices=N_CORES)

    xt = nc.dram_tensor("xt", [D, N], bf16, kind="ExternalInput")  # x[b].T
    wq = nc.dram_tensor("wq", [D, JL], bf16, kind="ExternalInput")  # folded+scaled
    wk = nc.dram_tensor("wk", [D, HD], bf16, kind="ExternalInput")  # folded
    wv = nc.dram_tensor("wv", [D, HD], bf16, kind="ExternalInput")
    wo = nc.dram_tensor("wo", [H * HD, DSLICE], bf16, kind="ExternalInput")
    cost = nc.dram_tensor("cost", [HD, N], bf16, kind="ExternalInput")
    # transposed output: outT[d, n]; host transposes back
    out = nc.dram_tensor("out", [DSLICE, N], f32, kind="ExternalOutput")

    xt_v = xt.rearrange("(ko p) n -> p ko n", p=128)
    wq_v = wq.rearrange("(ko p) j -> p ko j", p=128)
    wk_v = wk.rearrange("(ko p) j -> p ko j", p=128)
    wv_v = wv.rearrange("(ko p) j -> p ko j", p=128)
    wo_v = wo.rearrange("(ko p) d -> p ko d", p=128)

    from concourse.tile import add_dep_helper

    with tile.TileContext(nc) as tc:
        with (
            tc.tile_pool(name="big", bufs=1) as big_pool,
            tc.tile_pool(name="ag", bufs=3) as ag_pool,
            tc.tile_pool(name="otn", bufs=4) as otn_pool,
            tc.tile_pool(name="wpool", bufs=1) as w_pool,
            tc.tile_pool(name="work", bufs=1) as work_pool,
            tc.tile_pool(name="st", bufs=6) as st_pool,
            tc.tile_pool(name="tree", bufs=10) as tree_pool,
            tc.tile_pool(name="ev", bufs=2) as ev_pool,
            tc.tile_pool(name="psum", bufs=4, space="PSUM") as ps_pool,
            tc.tile_pool(name="psacc", bufs=2, space="PSUM") as psacc_pool,
            tc.tile_pool(name="dram", bufs=1, space="DRAM") as dram_pool,
        ):
            # ---- persistent SBUF tensors ----
            x_sb = big_pool.tile([128, KD, N], bf16, tag="big")
            wq_sb = w_pool.tile([128, KD, JL], bf16, tag="wq")
            wk_sb = w_pool.tile([128, KD, HD], bf16, tag="wk")
            wv_sb = w_pool.tile([128, KD, HD], bf16, tag="wv")
            wo_sb = w_pool.tile([128, KD, DSLICE], bf16, tag="wo")
            cos_sb = w_pool.tile([128, N], bf16, tag="cos")
            qT_sb = work_pool.tile([128, G, N], bf16, tag="qT")
            kT_sb = work_pool.tile([128, N], bf16, tag="kT")
            vT_sb = ag_pool.tile([128, HKV, N], bf16, tag="agsb", name="vT_sb")[:, 0, :]
            v_sb = work_pool.tile([128, N], bf16, tag="v")  # [m-part, mt*128+hd]
            ones_sb = work_pool.tile([128, 128], bf16, tag="ones")
            ident_sb = work_pool.tile([128, 128], bf16, tag="ident")

            nc.gpsimd.memset(ones_sb[:], 1.0)
            make_identity(nc, ident_sb[:])

            # ---- input DMAs (weights needed first come first), spread across
            # the two HWDGE queues (SP + Activation) ----
            nc.sync.dma_start(wk_sb[:], wk_v[:])
            nc.scalar.dma_start(cos_sb[:], cost[:, :])
            nc.scalar.dma_start(wv_sb[:], wv_v[:])
            # x in (ncx, kd) chunks so the first K-proj matmul starts after ~2MB
            for ncx in range(NC512):
                for kd in range(KD):
                    eng = nc.sync if kd % 2 == 0 else nc.scalar
                    eng.dma_start(x_sb[:, kd, ts(ncx, 512)], xt_v[:, kd, ts(ncx, 512)])
            nc.sync.dma_start(wq_sb[:], wq_v[:])
            nc.scalar.dma_start(wo_sb[:], wo_v[:])

            # ---- projections ----
            # k first (attention needs full kT before any head starts)
            for ncx in range(NC512):
                ps = ps_pool.tile([128, 512], f32, tag="mm")
                for kd in range(KD):
                    nc.tensor.matmul(
                        ps,
                        lhsT=wk_sb[:, kd, :],
                        rhs=x_sb[:, kd, ts(ncx, 512)],
                        start=(kd == 0),
                        stop=(kd == KD - 1),
                    )
                nc.vector.tensor_mul(kT_sb[:, ts(ncx, 512)], ps, cos_sb[:, ts(ncx, 512)])

            # v (as vT, then PE-transpose into natural [m, hd] layout)
            for ncx in range(NC512):
                ps = ps_pool.tile([128, 512], f32, tag="mm")
                for kd in range(KD):
                    nc.tensor.matmul(
                        ps,
                        lhsT=wv_sb[:, kd, :],
                        rhs=x_sb[:, kd, ts(ncx, 512)],
                        start=(kd == 0),
                        stop=(kd == KD - 1),
                    )
                nc.vector.tensor_copy(vT_sb[:, ts(ncx, 512)], ps)

            # q (4 heads); the V PE-transposes (which don't count as HAM
            # activity) are interleaved in small groups so the clock stays warm
            for h in range(G):
                for ncx in range(NC512):
                    ps = ps_pool.tile([128, 512], f32, tag="mm")
                    for kd in range(KD):
                        nc.tensor.matmul(
                            ps,
                            lhsT=wq_sb[:, kd, ts(h, 128)],
                            rhs=x_sb[:, kd, ts(ncx, 512)],
                            start=(kd == 0),
                            stop=(kd == KD - 1),
                        )
                    nc.vector.tensor_mul(
                        qT_sb[:, h, ts(ncx, 512)], ps, cos_sb[:, ts(ncx, 512)]
                    )
                    if h == 0:
                        q4 = ncx
                        ps_t = psacc_pool.tile([128, 512], bf16, tag="sums")
                        for j in range(4):
                            mt = q4 * 4 + j
                            nc.tensor.transpose(
                                ps_t[:, ts(j, 128)], vT_sb[:, ts(mt, 128)], ident_sb[:]
                            )
                        nc.vector.tensor_copy(v_sb[:, ts(q4, 512)], ps_t)

            # outT fp32 accumulator for the output projection (reuses x_sb's slot
            # footprint only after x is dead; separate tag keeps sizes honest)
            outT_acc = big_pool.tile([128, DC, N], f32, tag="big")

            # ---- attention + per-head chunked AllGather + Wo slab accumulation ----
            # heads 0-2: one gather over the full N; head 3 split 3+1 n-chunks so
            # the tail exposes only the small final chunk.
            ag_ins = []
            ag_outs = []
            for h in range(G - 1):
                ag_ins.append(
                    dram_pool.tile([HD, N], bf16, tag=f"agi{h}", name=f"agi{h}")
                )
                ag_outs.append(
                    dram_pool.tile(
                        [HKV * HD, N], bf16, tag=f"ago{h}", name=f"ago{h}"
                    )
                )
            ag_in3a = dram_pool.tile([HD, 1536], bf16, tag="agi3a", name="agi3a")
            ag_out3a = dram_pool.tile(
                [HKV * HD, 1536], bf16, tag="ago3a", name="ago3a"
            )
            ag_in3b = dram_pool.tile([HD, 512], bf16, tag="agi3b", name="agi3b")
            ag_out3b = dram_pool.tile(
                [HKV * HD, 512], bf16, tag="ago3b", name="ago3b"
            )

            # leading tiny collective: absorbs cross-core rendezvous skew while
            # the PE is busy with projections, so the first real gather is cheap
            bar_in = dram_pool.tile([1, 128], bf16, tag="bar_in", name="bar_in")
            bar_out = dram_pool.tile([4, 128], bf16, tag="bar_out", name="bar_out")
            nc.gpsimd.collective_compute(
                "AllGather",
                mybir.AluOpType.bypass,
                replica_groups=[[0, 1, 2, 3], [4, 5, 6, 7]],
                ins=[bar_in[:].opt()],
                outs=[bar_out[:].opt()],
            )

            attn_last = {}

            def attention_chunk(h, ncx):
                """One (head, 512-col n-chunk): scores pipelined one tile ahead
                of exp/AV; softmax sums via DVE pair tree + one ones-matmul."""
                ot_ps = psacc_pool.tile([128, 512], f32, tag="ot")
                s_pend = {}

                def emit_scores(mt):
                    ps = ps_pool.tile([128, 512], f32, tag="mm")
                    nc.tensor.matmul(
                        ps,
                        lhsT=kT_sb[:, ts(mt, 128)],
                        rhs=qT_sb[:, h, ts(ncx, 512)],
                        start=True,
                        stop=True,
                    )
                    s_pend[mt] = ps

                t1 = [None] * 8
                t2 = [None] * 4
                t3 = [None] * 2
                st_tiles = [None] * NT

                emit_scores(0)
                for mt in range(NT):
                    if mt + 1 < NT:
                        emit_scores(mt + 1)
                    st_sb = st_pool.tile([128, 512], bf16, tag="st")
                    nc.scalar.activation(st_sb[:], s_pend.pop(mt), AFT.Exp)
                    st_tiles[mt] = st_sb
                    nc.tensor.matmul(
                        ot_ps,
                        lhsT=v_sb[:, ts(mt, 128)],
                        rhs=st_sb[:],
                        start=(mt == 0),
                        stop=(mt == NT - 1),
                    )
                    # DVE pair-tree for softmax denominators
                    if mt % 2 == 1:
                        j = mt // 2
                        t1[j] = tree_t = tree_pool.tile(
                            [128, 512], bf16, tag="tr", name=f"t1_{h}_{ncx}_{j}"
                        )
                        nc.vector.tensor_add(
                            tree_t[:], st_tiles[mt - 1][:], st_tiles[mt][:]
                        )
                        st_tiles[mt - 1] = st_tiles[mt] = None
                        if j % 2 == 1:
                            j2 = j // 2
                            t2[j2] = tree_t = tree_pool.tile(
                                [128, 512], bf16, tag="tr", name=f"t2_{h}_{ncx}_{j2}"
                            )
                            nc.vector.tensor_add(
                                tree_t[:], t1[j - 1][:], t1[j][:]
                            )
                            t1[j - 1] = t1[j] = None
                            if j2 % 2 == 1:
                                j3 = j2 // 2
                                t3[j3] = tree_t = tree_pool.tile(
                                    [128, 512],
                                    bf16,
                                    tag="tr",
                                    name=f"t3_{h}_{ncx}_{j3}",
                                )
                                nc.vector.tensor_add(
                                    tree_t[:], t2[j2 - 1][:], t2[j2][:]
                                )
                                t2[j2 - 1] = t2[j2] = None
                acc_sb = tree_pool.tile([128, 512], bf16, tag="tr")
                nc.vector.tensor_add(acc_sb[:], t3[0][:], t3[1][:])
                sums_ps = psacc_pool.tile([128, 512], f32, tag="sums")
                sums_mm = nc.tensor.matmul(
                    sums_ps, lhsT=ones_sb[:], rhs=acc_sb[:], start=True, stop=True
                )
                attn_last[h] = sums_mm
                recip_sb = ev_pool.tile([128, 512], f32, tag="recip")
                nc.vector.reciprocal_approx_fast(recip_sb[:], sums_ps)
                otn_sb = otn_pool.tile([128, 512], bf16, tag="otn")
                nc.vector.tensor_mul(otn_sb[:], ot_ps, recip_sb[:])
                return otn_sb

            def slab_contribution(h, ag_sb, nns, gate, out_cols=None):
                """Add gathered head-h slab's term (n-chunks nns) to outT_acc.

                out_cols=(lo, hi): after each dc's adds, fire the out DMA for
                that column range (queues alternate so the tail isn't serial).
                """
                for dc in range(DC):
                    for nn in nns:
                        off = nn - nns[0]
                        ps = ps_pool.tile([128, 512], f32, tag="mm", name=f"sd{h}")
                        for r in range(HKV):
                            jc = r * G + h
                            mm = nc.tensor.matmul(
                                ps,
                                lhsT=wo_sb[:, jc, ts(dc, 128)],
                                rhs=ag_sb[:, r, ts(off, 512)],
                                start=(r == 0),
                                stop=(r == HKV - 1),
                            )
                            if gate is not None:
                                # placement hint: keep slab matmuls out of the
                                # PE stream until the gate head's attention is
                                # done, so the PE never waits on a gather
                                add_dep_helper(
                                    mm.ins,
                                    gate.ins,
                                    sync=True,
                                    reason="slab after later attention",
                                )
                        if h == 0:
                            nc.vector.tensor_copy(outT_acc[:, dc, ts(nn, 512)], ps)
                        else:
                            nc.vector.tensor_add(
                                outT_acc[:, dc, ts(nn, 512)],
                                ps,
                                outT_acc[:, dc, ts(nn, 512)],
                            )
                    if out_cols is not None:
                        lo, hi = out_cols
                        eng = nc.sync if dc % 2 == 0 else nc.scalar
                        eng.dma_start(
                            out[ts(dc, 128), lo:hi], outT_acc[:, dc, lo:hi]
                        )

            def gathered_sb(h, ag_out, ncols, gate, split_r=False):
                ag_v = ag_out.rearrange("(r p) n -> p r n", p=128)
                ag_sb = ag_pool.tile(
                    [128, HKV, ncols], bf16, tag="agsb", name=f"agsb{h}"
                )
                if split_r:
                    for r in range(HKV):
                        eng = nc.sync if r % 2 == 0 else nc.scalar
                        ag_dma = eng.dma_start(ag_sb[:, r, :], ag_v[:, r, :])
                        if gate is not None:
                            add_dep_helper(
                                ag_dma.ins,
                                gate.ins,
                                sync=True,
                                reason="keep slab DMA behind later otn DMAs",
                            )
                else:
                    ag_dma = nc.sync.dma_start(ag_sb[:], ag_v[:])
                    if gate is not None:
                        add_dep_helper(
                            ag_dma.ins,
                            gate.ins,
                            sync=True,
                            reason="keep slab DMA behind later otn DMAs",
                        )
                return ag_sb

            rg = [[0, 1, 2, 3], [4, 5, 6, 7]]
            for h in range(G):
                for ncx in range(NC512):
                    otn_sb = attention_chunk(h, ncx)
                    if h < G - 1:
                        nc.sync.dma_start(ag_ins[h][:, ts(ncx, 512)], otn_sb[:])
                    elif ncx < 3:
                        nc.sync.dma_start(ag_in3a[:, ts(ncx, 512)], otn_sb[:])
                    else:
                        nc.sync.dma_start(ag_in3b[:], otn_sb[:])
                    if h == G - 1 and ncx == 2:
                        nc.gpsimd.collective_compute(
                            "AllGather",
                            mybir.AluOpType.bypass,
                            replica_groups=rg,
                            ins=[ag_in3a[:].opt()],
                            outs=[ag_out3a[:].opt()],
                        )
                if h < G - 1:
                    nc.gpsimd.collective_compute(
                        "AllGather",
                        mybir.AluOpType.bypass,
                        replica_groups=rg,
                        ins=[ag_ins[h][:].opt()],
                        outs=[ag_outs[h][:].opt()],
                    )
                else:
                    nc.gpsimd.collective_compute(
                        "AllGather",
                        mybir.AluOpType.bypass,
                        replica_groups=rg,
                        ins=[ag_in3b[:].opt()],
                        outs=[ag_out3b[:].opt()],
                    )

            # slab matmuls after all attention: gathers 0-2 hide under attention;
            # gather 3a hides under slabs 0-2; only the small 3b chunk is tail
            for h in range(G - 1):
                gate = attn_last[min(h + 1, G - 1)]
                ag_sb = gathered_sb(h, ag_outs[h], N, gate)
                slab_contribution(h, ag_sb, list(range(NC512)), gate)
            gate = attn_last[G - 1]
            ag_sb3a = gathered_sb(3, ag_out3a, 1536, gate, split_r=True)
            slab_contribution(3, ag_sb3a, [0, 1, 2], gate, out_cols=(0, 1536))
            ag_sb3b = gathered_sb(3, ag_out3b, 512, gate, split_r=True)
            slab_contribution(3, ag_sb3b, [3], gate, out_cols=(1536, 2048))

    nc.compile()
    _NC_CACHE["nc"] = nc
    return nc


def kernel(x, Wq, Wk, Wv, Wo):
    _install_axon_ntff_hook()
    import ml_dtypes

    import concourse.bass_utils as bass_utils

    bass_utils.upload_artifacts = lambda tmpdir: str(tmpdir)
    from concourse.bass_utils import run_bass_kernel_spmd

    x = np.asarray(x, dtype=np.float32)
    Wq = np.asarray(Wq, dtype=np.float32)
    Wk = np.asarray(Wk, dtype=np.float32)
    Wv = np.asarray(Wv, dtype=np.float32)
    Wo = np.asarray(Wo, dtype=np.float32)

    bf = ml_dtypes.bfloat16
    scale = np.float32(HD**-0.5)
    wq_f = (_fold_rope(Wq, H) * scale).astype(bf)  # [D, 2048]
    wk_f = _fold_rope(Wk, HKV).astype(bf)  # [D, 512]
    wv_f = Wv.astype(bf)  # [D, 512]
    wo_f = Wo.astype(bf)  # [2048, D]
    cos_t = _cos_table().astype(bf)  # [128, N]

    xt = [np.ascontiguousarray(x[b].T).astype(bf) for b in range(B)]

    in_maps = []
    for c in range(N_CORES):
        b, g = divmod(c, HKV)
        in_maps.append(
            {
                "xt": xt[b],
                "wq": np.ascontiguousarray(wq_f[:, g * JL : (g + 1) * JL]),
                "wk": np.ascontiguousarray(wk_f[:, g * HD : (g + 1) * HD]),
                "wv": np.ascontiguousarray(wv_f[:, g * HD : (g + 1) * HD]),
                "wo": np.ascontiguousarray(wo_f[:, g * DSLICE : (g + 1) * DSLICE]),
                "cost": cos_t,
            }
        )

    nc = _build_nc()
    res = run_bass_kernel_spmd(nc, in_maps, list(range(N_CORES)))

    out = np.empty((B, N, D), dtype=np.float32)
    for c in range(N_CORES):
        b, g = divmod(c, HKV)
        out[b, :, g * DSLICE : (g + 1) * DSLICE] = res.results[c]["out"].T
    return out


# revision 8
# speedup vs baseline: 1.0344x; 1.0344x over previous
"""GroupedQueryAttention (B=2, N=2048, D=2048, H=16, HKV=4, HD=128) on 8 trn2 cores.

Sharding: core c handles (batch b = c//4, kv-head g = c%4): 4 q-heads + 1 kv head.
RoPE (with the reference's sin==cos quirk) is folded into Wq/Wk host-side:
  q_rot = cos ⊙ (M q) with constant M = [[I,-I],[I,I]] acting on head-dim halves,
so on-device RoPE is just an elementwise multiply by a precomputed cos table.
The softmax scale is folded into Wq. All matmuls run in bf16 with fp32 PSUM
accumulation. Attention uses a transpose-free layout chain:
  qT,kT [hd,n]; ST = kT.T @ qT [m,n]; exp on ScalarE; OT += v.T@ST (v as [m,hd]).
The scores loop is software-pipelined one tile ahead so the PE never stalls on
the ScalarE exp. Softmax denominators: DVE pairwise-add tree over the 16 exp
tiles, then a single all-ones stationary matmul per (head, n-chunk); normalize
with DVE fast-reciprocal. Per-head chunked AllGathers (bf16) overlap with
attention of later heads (head 3's gather is split 3+1 n-chunks to shrink the
tail); the output projection accumulates per gathered slab into an SBUF fp32
accumulator in transposed [d, n] layout (Wo chunks stationary), and the host
transposes each core's [512, 2048] slice back.
Host gathers: out[b][:, g*512:(g+1)*512] = core (b,g) output transposed.
"""

import sys
import types

import numpy as np

B, N, D = 2, 2048, 2048
H, HKV, HD = 16, 4, 128
G = H // HKV  # q heads per kv head = 4
N_CORES = 8
ROPE_BASE = 10000.0
DSLICE = D // G  # 512 output columns per core
JL = G * HD  # 512 local attention-output rows per core


def _install_axon_ntff_hook():
    """This container's antenv lacks axon_hooks; inject it so trace=True works."""
    if "antenv.axon_hooks" in sys.modules:
        return
    try:
        from trn_agent_boot.trn_boot import _ntff_profile_via_ctypes

        hook = _ntff_profile_via_ctypes("/opt/axon/libaxon_pjrt.so")
    except Exception:
        hook = None
    mod = types.ModuleType("antenv.axon_hooks")
    mod.get_axon_ntff_profile_hook = lambda: hook
    mod.set_axon_ntff_profile_hook = lambda h: None
    sys.modules["antenv.axon_hooks"] = mod


def _fold_rope(w: np.ndarray, n_heads: int) -> np.ndarray:
    """Return W' with the (sin==cos) RoPE mixing folded in: x@W' = M(x@W) per head."""
    wf = w.reshape(D, n_heads, HD)
    lo, hi = wf[..., : HD // 2], wf[..., HD // 2 :]
    return np.concatenate([lo - hi, hi + lo], axis=-1).reshape(D, n_heads * HD)


def _cos_table() -> np.ndarray:
    inv_freq = 1.0 / (ROPE_BASE ** (np.arange(0, HD, 2, dtype=np.float64) / HD))
    freqs = np.arange(N, dtype=np.float64)[:, None] * inv_freq[None, :]  # [N, 64]
    emb = np.concatenate([freqs, freqs], axis=-1)  # [N, 128]
    return np.cos(emb).T.astype(np.float32).copy()  # [128, N]


_NC_CACHE: dict = {}


def _build_nc():
    if "nc" in _NC_CACHE:
        return _NC_CACHE["nc"]

    import concourse.bacc as bacc
    import concourse.mybir as mybir
    import concourse.tile as tile
    from concourse.bass import ts
    from concourse.masks import make_identity

    f32 = mybir.dt.float32
    bf16 = mybir.dt.bfloat16
    AFT = mybir.ActivationFunctionType
    KD = D // 128  # 16 contraction chunks
    NT = N // 128  # 16 m tiles of 128
    NC512 = N // 512  # 4 chunks of 512
    DC = DSLICE // 128  # 4 output-column chunks of 128

    nc = bacc.Bacc(target_bir_lowering=False, debug=False, num_devices=N_CORES)

    xt = nc.dram_tensor("xt", [D, N], bf16, kind="ExternalInput")  # x[b].T
    wq = nc.dram_tensor("wq", [D, JL], bf16, kind="ExternalInput")  # folded+scaled
    wk = nc.dram_tensor("wk", [D, HD], bf16, kind="ExternalInput")  # folded
    wv = nc.dram_tensor("wv", [D, HD], bf16, kind="ExternalInput")
    wo = nc.dram_tensor("wo", [H * HD, DSLICE], bf16, kind="ExternalInput")
    cost = nc.dram_tensor("cost", [HD, N], bf16, kind="ExternalInput")
    # transposed output: outT[d, n]; host transposes back
    out = nc.dram_tensor("out", [DSLICE, N], f32, kind="ExternalOutput")

    xt_v = xt.rearrange("(ko p) n -> p ko n", p=128)
    wq_v = wq.rearrange("(ko p) j -> p ko j", p=128)
    wk_v = wk.rearrange("(ko p) j -> p ko j", p=128)
    wv_v = wv.rearrange("(ko p) j -> p ko j", p=128)
    wo_v = wo.rearrange("(ko p) d -> p ko d", p=128)

    from concourse.tile import add_dep_helper

    with tile.TileContext(nc) as tc:
        with (
            tc.tile_pool(name="big", bufs=1) as big_pool,
            tc.tile_pool(name="ag", bufs=3) as ag_pool,
            tc.tile_pool(name="otn", bufs=4) as otn_pool,
            tc.tile_pool(name="wpool", bufs=1) as w_pool,
            tc.tile_pool(name="work", bufs=1) as work_pool,
            tc.tile_pool(name="st", bufs=6) as st_pool,
            tc.tile_pool(name="tree", bufs=10) as tree_pool,
            tc.tile_pool(name="ev", bufs=2) as ev_pool,
            tc.tile_pool(name="psum", bufs=4, space="PSUM") as ps_pool,
            tc.tile_pool(name="psacc", bufs=2, space="PSUM") as psacc_pool,
            tc.tile_pool(name="dram", bufs=1, space="DRAM") as dram_pool,
        ):
            # ---- persistent SBUF tensors ----
            x_sb = big_pool.tile([128, KD, N], bf16, tag="big")
            wq_sb = w_pool.tile([128, KD, JL], bf16, tag="wq")
            wk_sb = w_pool.tile([128, KD, HD], bf16, tag="wk")
            wv_sb = w_pool.tile([128, KD, HD], bf16, tag="wv")
            wo_sb = w_pool.tile([128, KD, DSLICE], bf16, tag="wo")
            cos_sb = w_pool.tile([128, N], bf16, tag="cos")
            qT_sb = work_pool.tile([128, G, N], bf16, tag="qT")
            kT_sb = work_pool.tile([128, N], bf16, tag="kT")
            vT_sb = ag_pool.tile([128, HKV, N], bf16, tag="agsb", name="vT_sb")[:, 0, :]
            v_sb = work_pool.tile([128, N], bf16, tag="v")  # [m-part, mt*128+hd]
            ones_sb = work_pool.tile([128, 128], bf16, tag="ones")
            ident_sb = work_pool.tile([128, 128], bf16, tag="ident")

            nc.gpsimd.memset(ones_sb[:], 1.0)
            make_identity(nc, ident_sb[:])

            # ---- input DMAs (weights needed first come first) ----
            # x rows stay full-width (4KB partition lines — chunking them into
            # 512-col pieces drops DMA throughput to ~160GB/s); the sync queue
            # carries wk + x so nothing slow-lined sits ahead of the x stream.
            nc.sync.dma_start(wk_sb[:], wk_v[:])
            nc.scalar.dma_start(cos_sb[:], cost[:, :])
            nc.scalar.dma_start(wv_sb[:], wv_v[:])
            for kd in range(KD):
                nc.sync.dma_start(x_sb[:, kd, :], xt_v[:, kd, :])
            nc.scalar.dma_start(wq_sb[:], wq_v[:])
            nc.scalar.dma_start(wo_sb[:], wo_v[:])

            # ---- projections ----
            # k first (attention needs full kT before any head starts);
            # kd-outer over 4 live PSUM banks so the PE consumes x rows as
            # they land instead of waiting for the full 8MB
            kps = [
                ps_pool.tile([128, 512], f32, tag="mm", name=f"kps{ncx}")
                for ncx in range(NC512)
            ]
            for kd in range(KD):
                for ncx in range(NC512):
                    nc.tensor.matmul(
                        kps[ncx],
                        lhsT=wk_sb[:, kd, :],
                        rhs=x_sb[:, kd, ts(ncx, 512)],
                        start=(kd == 0),
                        stop=(kd == KD - 1),
                    )
            for ncx in range(NC512):
                nc.vector.tensor_mul(
                    kT_sb[:, ts(ncx, 512)], kps[ncx], cos_sb[:, ts(ncx, 512)]
                )

            # v (as vT, then PE-transpose into natural [m, hd] layout)
            for ncx in range(NC512):
                ps = ps_pool.tile([128, 512], f32, tag="mm")
                for kd in range(KD):
                    nc.tensor.matmul(
                        ps,
                        lhsT=wv_sb[:, kd, :],
                        rhs=x_sb[:, kd, ts(ncx, 512)],
                        start=(kd == 0),
                        stop=(kd == KD - 1),
                    )
                nc.vector.tensor_copy(vT_sb[:, ts(ncx, 512)], ps)

            # q (4 heads); the V PE-transposes (which don't count as HAM
            # activity) are interleaved in small groups so the clock stays warm
            for h in range(G):
                for ncx in range(NC512):
                    ps = ps_pool.tile([128, 512], f32, tag="mm")
                    for kd in range(KD):
                        nc.tensor.matmul(
                            ps,
                            lhsT=wq_sb[:, kd, ts(h, 128)],
                            rhs=x_sb[:, kd, ts(ncx, 512)],
                            start=(kd == 0),
                            stop=(kd == KD - 1),
                        )
                    nc.vector.tensor_mul(
                        qT_sb[:, h, ts(ncx, 512)], ps, cos_sb[:, ts(ncx, 512)]
                    )
                    if h == 0:
                        q4 = ncx
                        ps_t = psacc_pool.tile([128, 512], bf16, tag="sums")
                        for j in range(4):
                            mt = q4 * 4 + j
                            nc.tensor.transpose(
                                ps_t[:, ts(j, 128)], vT_sb[:, ts(mt, 128)], ident_sb[:]
                            )
                        nc.vector.tensor_copy(v_sb[:, ts(q4, 512)], ps_t)

            # outT fp32 accumulator for the output projection (reuses x_sb's slot
            # footprint only after x is dead; separate tag keeps sizes honest)
            outT_acc = big_pool.tile([128, DC, N], f32, tag="big")

            # ---- attention + per-head chunked AllGather + Wo slab accumulation ----
            # heads 0-2: one gather over the full N; head 3 split 3+1 n-chunks so
            # the tail exposes only the small final chunk.
            ag_ins = []
            ag_outs = []
            for h in range(G - 1):
                ag_ins.append(
                    dram_pool.tile([HD, N], bf16, tag=f"agi{h}", name=f"agi{h}")
                )
                ag_outs.append(
                    dram_pool.tile(
                        [HKV * HD, N], bf16, tag=f"ago{h}", name=f"ago{h}"
                    )
                )
            ag_in3a = dram_pool.tile([HD, 1536], bf16, tag="agi3a", name="agi3a")
            ag_out3a = dram_pool.tile(
                [HKV * HD, 1536], bf16, tag="ago3a", name="ago3a"
            )
            ag_in3b = dram_pool.tile([HD, 512], bf16, tag="agi3b", name="agi3b")
            ag_out3b = dram_pool.tile(
                [HKV * HD, 512], bf16, tag="ago3b", name="ago3b"
            )

            # leading tiny collective: absorbs cross-core rendezvous skew while
            # the PE is busy with projections, so the first real gather is cheap
            bar_in = dram_pool.tile([1, 128], bf16, tag="bar_in", name="bar_in")
            bar_out = dram_pool.tile([4, 128], bf16, tag="bar_out", name="bar_out")
            nc.gpsimd.collective_compute(
                "AllGather",
                mybir.AluOpType.bypass,
                replica_groups=[[0, 1, 2, 3], [4, 5, 6, 7]],
                ins=[bar_in[:].opt()],
                outs=[bar_out[:].opt()],
            )

            attn_last = {}

            def attention_chunk(h, ncx):
                """One (head, 512-col n-chunk): scores pipelined one tile ahead
                of exp/AV; softmax sums via DVE pair tree + one ones-matmul."""
                ot_ps = psacc_pool.tile([128, 512], f32, tag="ot")
                s_pend = {}

                def emit_scores(mt):
                    ps = ps_pool.tile([128, 512], f32, tag="mm")
                    nc.tensor.matmul(
                        ps,
                        lhsT=kT_sb[:, ts(mt, 128)],
                        rhs=qT_sb[:, h, ts(ncx, 512)],
                        start=True,
                        stop=True,
                    )
                    s_pend[mt] = ps

                t1 = [None] * 8
                t2 = [None] * 4
                t3 = [None] * 2
                st_tiles = [None] * NT

                emit_scores(0)
                for mt in range(NT):
                    if mt + 1 < NT:
                        emit_scores(mt + 1)
                    st_sb = st_pool.tile([128, 512], bf16, tag="st")
                    nc.scalar.activation(st_sb[:], s_pend.pop(mt), AFT.Exp)
                    st_tiles[mt] = st_sb
                    nc.tensor.matmul(
                        ot_ps,
                        lhsT=v_sb[:, ts(mt, 128)],
                        rhs=st_sb[:],
                        start=(mt == 0),
                        stop=(mt == NT - 1),
                    )
                    # DVE pair-tree for softmax denominators
                    if mt % 2 == 1:
                        j = mt // 2
                        t1[j] = tree_t = tree_pool.tile(
                            [128, 512], bf16, tag="tr", name=f"t1_{h}_{ncx}_{j}"
                        )
                        nc.vector.tensor_add(
                            tree_t[:], st_tiles[mt - 1][:], st_tiles[mt][:]
                        )
                        st_tiles[mt - 1] = st_tiles[mt] = None
                        if j % 2 == 1:
                            j2 = j // 2
                            t2[j2] = tree_t = tree_pool.tile(
                                [128, 512], bf16, tag="tr", name=f"t2_{h}_{ncx}_{j2}"
                            )
                            nc.vector.tensor_add(
                                tree_t[:], t1[j - 1][:], t1[j][:]
                            )
                            t1[j - 1] = t1[j] = None
                            if j2 % 2 == 1:
                                j3 = j2 // 2
                                t3[j3] = tree_t = tree_pool.tile(
                                    [128, 512],
                                    bf16,
                                    tag="tr",
                                    name=f"t3_{h}_{ncx}_{j3}",
                                )
                                nc.vector.tensor_add(
                                    tree_t[:], t2[j2 - 1][:], t2[j2][:]
                                )
                                t2[j2 - 1] = t2[j2] = None
                acc_sb = tree_pool.tile([128, 512], bf16, tag="tr")
                nc.vector.tensor_add(acc_sb[:], t3[0][:], t3[1][:])
                sums_ps = psacc_pool.tile([128, 512], f32, tag="sums")
                sums_mm = nc.tensor.matmul(
                    sums_ps, lhsT=ones_sb[:], rhs=acc_sb[:], start=True, stop=True
                )
                attn_last[h] = sums_mm
                recip_sb = ev_pool.tile([128, 512], f32, tag="recip")
                nc.vector.reciprocal_approx_fast(recip_sb[:], sums_ps)
                otn_sb = otn_pool.tile([128, 512], bf16, tag="otn")
                nc.vector.tensor_mul(otn_sb[:], ot_ps, recip_sb[:])
                return otn_sb

            def slab_contribution(h, ag_sb, nns, gate, out_cols=None):
                """Add gathered head-h slab's term (n-chunks nns) to outT_acc.

                out_cols=(lo, hi): after each dc's adds, fire the out DMA for
                that column range (queues alternate so the tail isn't serial).
                """
                for dc in range(DC):
                    for nn in nns:
                        off = nn - nns[0]
                        ps = ps_pool.tile([128, 512], f32, tag="mm", name=f"sd{h}")
                        for r in range(HKV):
                            jc = r * G + h
                            mm = nc.tensor.matmul(
                                ps,
                                lhsT=wo_sb[:, jc, ts(dc, 128)],
                                rhs=ag_sb[:, r, ts(off, 512)],
                                start=(r == 0),
                                stop=(r == HKV - 1),
                            )
                            if gate is not None:
                                # placement hint: keep slab matmuls out of the
                                # PE stream until the gate head's attention is
                                # done, so the PE never waits on a gather
                                add_dep_helper(
                                    mm.ins,
                                    gate.ins,
                                    sync=True,
                                    reason="slab after later attention",
                                )
                        if h == 0:
                            nc.vector.tensor_copy(outT_acc[:, dc, ts(nn, 512)], ps)
                        else:
                            nc.vector.tensor_add(
                                outT_acc[:, dc, ts(nn, 512)],
                                ps,
                                outT_acc[:, dc, ts(nn, 512)],
                            )
                    if out_cols is not None:
                        lo, hi = out_cols
                        eng = nc.sync if dc % 2 == 0 else nc.scalar
                        eng.dma_start(
                            out[ts(dc, 128), lo:hi], outT_acc[:, dc, lo:hi]
                        )

            def gathered_sb(h, ag_out, ncols, gate, split_r=False):
                ag_v = ag_out.rearrange("(r p) n -> p r n", p=128)
                ag_sb = ag_pool.tile(
                    [128, HKV, ncols], bf16, tag="agsb", name=f"agsb{h}"
                )
                if split_r:
                    for r in range(HKV):
                        eng = nc.sync if r % 2 == 0 else nc.scalar
                        ag_dma = eng.dma_start(ag_sb[:, r, :], ag_v[:, r, :])
                        if gate is not None:
                            add_dep_helper(
                                ag_dma.ins,
                                gate.ins,
                                sync=True,
                                reason="keep slab DMA behind later otn DMAs",
                            )
                else:
                    ag_dma = nc.sync.dma_start(ag_sb[:], ag_v[:])
                    if gate is not None:
                        add_dep_helper(
                            ag_dma.ins,
                            gate.ins,
                            sync=True,
                            reason="keep slab DMA behind later otn DMAs",
                        )
                return ag_sb

            rg = [[0, 1, 2, 3], [4, 5, 6, 7]]
            for h in range(G):
                for ncx in range(NC512):
                    otn_sb = attention_chunk(h, ncx)
                    if h < G - 1:
                        nc.sync.dma_start(ag_ins[h][:, ts(ncx, 512)], otn_sb[:])
                    elif ncx < 3:
                        nc.sync.dma_start(ag_in3a[:, ts(ncx, 512)], otn_sb[:])
                    else:
                        nc.sync.dma_start(ag_in3b[:], otn_sb[:])
                    if h == G - 1 and ncx == 2:
                        nc.gpsimd.collective_compute(
                            "AllGather",
                            mybir.AluOpType.bypass,
                            replica_groups=rg,
                            ins=[ag_in3a[:].opt()],
                            outs=[ag_out3a[:].opt()],
                        )
                if h < G - 1:
                    nc.gpsimd.collective_compute(
                        "AllGather",
                        mybir.AluOpType.bypass,
                        replica_groups=rg,
                        ins=[ag_ins[h][:].opt()],
                        outs=[ag_outs[h][:].opt()],
                    )
                else:
                    nc.gpsimd.collective_compute(
                        "AllGather",
                        mybir.AluOpType.bypass,
                        replica_groups=rg,
                        ins=[ag_in3b[:].opt()],
                        outs=[ag_out3b[:].opt()],
                    )

            # slab matmuls after all attention: gathers 0-2 hide under attention;
            # gather 3a hides under slabs 0-2; only the small 3b chunk is tail
            for h in range(G - 1):
                gate = attn_last[min(h + 1, G - 1)]
                ag_sb = gathered_sb(h, ag_outs[h], N, gate)
                slab_contribution(h, ag_sb, list(range(NC512)), gate)
            gate = attn_last[G - 1]
            ag_sb3a = gathered_sb(3, ag_out3a, 1536, gate, split_r=True)
            slab_contribution(3, ag_sb3a, [0, 1, 2], gate, out_cols=(0, 1536))
            ag_sb3b = gathered_sb(3, ag_out3b, 512, gate, split_r=True)
            slab_contribution(3, ag_sb3b, [3], gate, out_cols=(1536, 2048))

    nc.compile()
    _NC_CACHE["nc"] = nc
    return nc


def kernel(x, Wq, Wk, Wv, Wo):
    _install_axon_ntff_hook()
    import ml_dtypes

    import concourse.bass_utils as bass_utils

    bass_utils.upload_artifacts = lambda tmpdir: str(tmpdir)
    from concourse.bass_utils import run_bass_kernel_spmd

    x = np.asarray(x, dtype=np.float32)
    Wq = np.asarray(Wq, dtype=np.float32)
    Wk = np.asarray(Wk, dtype=np.float32)
    Wv = np.asarray(Wv, dtype=np.float32)
    Wo = np.asarray(Wo, dtype=np.float32)

    bf = ml_dtypes.bfloat16
    scale = np.float32(HD**-0.5)
    wq_f = (_fold_rope(Wq, H) * scale).astype(bf)  # [D, 2048]
    wk_f = _fold_rope(Wk, HKV).astype(bf)  # [D, 512]
    wv_f = Wv.astype(bf)  # [D, 512]
    wo_f = Wo.astype(bf)  # [2048, D]
    cos_t = _cos_table().astype(bf)  # [128, N]

    xt = [np.ascontiguousarray(x[b].T).astype(bf) for b in range(B)]

    in_maps = []
    for c in range(N_CORES):
        b, g = divmod(c, HKV)
        in_maps.append(
            {
                "xt": xt[b],
                "wq": np.ascontiguousarray(wq_f[:, g * JL : (g + 1) * JL]),
                "wk": np.ascontiguousarray(wk_f[:, g * HD : (g + 1) * HD]),
                "wv": np.ascontiguousarray(wv_f[:, g * HD : (g + 1) * HD]),
                "wo": np.ascontiguousarray(wo_f[:, g * DSLICE : (g + 1) * DSLICE]),
                "cost": cos_t,
            }
        )

    nc = _build_nc()
    res = run_bass_kernel_spmd(nc, in_maps, list(range(N_CORES)))

    out = np.empty((B, N, D), dtype=np.float32)
    for c in range(N_CORES):
        b, g = divmod(c, HKV)
        out[b, :, g * DSLICE : (g + 1) * DSLICE] = res.results[c]["out"].T
    return out


# revision 9
# speedup vs baseline: 1.0590x; 1.0237x over previous
"""GroupedQueryAttention (B=2, N=2048, D=2048, H=16, HKV=4, HD=128) on 8 trn2 cores.

Sharding: core c handles (batch b = c//4, kv-head g = c%4): 4 q-heads + 1 kv head.
RoPE (with the reference's sin==cos quirk) is folded into Wq/Wk host-side:
  q_rot = cos ⊙ (M q) with constant M = [[I,-I],[I,I]] acting on head-dim halves,
so on-device RoPE is just an elementwise multiply by a precomputed cos table.
The softmax scale is folded into Wq. All matmuls run in bf16 with fp32 PSUM
accumulation. Attention uses a transpose-free layout chain:
  qT,kT [hd,n]; ST = kT.T @ qT [m,n]; exp on ScalarE; OT += v.T@ST (v as [m,hd]).
The scores loop is software-pipelined one tile ahead so the PE never stalls on
the ScalarE exp. Softmax denominators: DVE pairwise-add tree over the 16 exp
tiles, then a single all-ones stationary matmul per (head, n-chunk); normalize
with DVE fast-reciprocal. Per-head chunked AllGathers (bf16) overlap with
attention of later heads (head 3's gather is split 3+1 n-chunks to shrink the
tail); the output projection accumulates per gathered slab into an SBUF fp32
accumulator in transposed [d, n] layout (Wo chunks stationary), and the host
transposes each core's [512, 2048] slice back.
Host gathers: out[b][:, g*512:(g+1)*512] = core (b,g) output transposed.
"""

import sys
import types

import numpy as np

B, N, D = 2, 2048, 2048
H, HKV, HD = 16, 4, 128
G = H // HKV  # q heads per kv head = 4
N_CORES = 8
ROPE_BASE = 10000.0
DSLICE = D // G  # 512 output columns per core
JL = G * HD  # 512 local attention-output rows per core


def _install_axon_ntff_hook():
    """This container's antenv lacks axon_hooks; inject it so trace=True works."""
    if "antenv.axon_hooks" in sys.modules:
        return
    try:
        from trn_agent_boot.trn_boot import _ntff_profile_via_ctypes

        hook = _ntff_profile_via_ctypes("/opt/axon/libaxon_pjrt.so")
    except Exception:
        hook = None
    mod = types.ModuleType("antenv.axon_hooks")
    mod.get_axon_ntff_profile_hook = lambda: hook
    mod.set_axon_ntff_profile_hook = lambda h: None
    sys.modules["antenv.axon_hooks"] = mod


def _fold_rope(w: np.ndarray, n_heads: int) -> np.ndarray:
    """Return W' with the (sin==cos) RoPE mixing folded in: x@W' = M(x@W) per head."""
    wf = w.reshape(D, n_heads, HD)
    lo, hi = wf[..., : HD // 2], wf[..., HD // 2 :]
    return np.concatenate([lo - hi, hi + lo], axis=-1).reshape(D, n_heads * HD)


def _cos_table() -> np.ndarray:
    inv_freq = 1.0 / (ROPE_BASE ** (np.arange(0, HD, 2, dtype=np.float64) / HD))
    freqs = np.arange(N, dtype=np.float64)[:, None] * inv_freq[None, :]  # [N, 64]
    emb = np.concatenate([freqs, freqs], axis=-1)  # [N, 128]
    return np.cos(emb).T.astype(np.float32).copy()  # [128, N]


_NC_CACHE: dict = {}


def _build_nc():
    if "nc" in _NC_CACHE:
        return _NC_CACHE["nc"]

    import concourse.bacc as bacc
    import concourse.mybir as mybir
    import concourse.tile as tile
    from concourse.bass import ts
    from concourse.masks import make_identity

    f32 = mybir.dt.float32
    bf16 = mybir.dt.bfloat16
    AFT = mybir.ActivationFunctionType
    KD = D // 128  # 16 contraction chunks
    NT = N // 128  # 16 m tiles of 128
    NC512 = N // 512  # 4 chunks of 512
    DC = DSLICE // 128  # 4 output-column chunks of 128

    nc = bacc.Bacc(target_bir_lowering=False, debug=False, num_devices=N_CORES)

    xt = nc.dram_tensor("xt", [D, N], bf16, kind="ExternalInput")  # x[b].T
    # weights arrive pre-rearranged host-side to [p, ko, cols] so their DMAs
    # have long contiguous partition lines (the (ko p) row-interleaved view
    # yields 256B lines and ~5x slower DMA)
    wq = nc.dram_tensor("wq", [128, KD * JL], bf16, kind="ExternalInput")
    wk = nc.dram_tensor("wk", [128, KD * HD], bf16, kind="ExternalInput")
    wv = nc.dram_tensor("wv", [128, KD * HD], bf16, kind="ExternalInput")
    wo = nc.dram_tensor("wo", [128, KD * DSLICE], bf16, kind="ExternalInput")
    cost = nc.dram_tensor("cost", [HD, N], bf16, kind="ExternalInput")
    # transposed output: outT[d, n]; host transposes back
    out = nc.dram_tensor("out", [DSLICE, N], f32, kind="ExternalOutput")

    xt_v = xt.rearrange("(ko p) n -> p ko n", p=128)
    wq_v = wq.rearrange("p (ko j) -> p ko j", ko=KD)
    wk_v = wk.rearrange("p (ko j) -> p ko j", ko=KD)
    wv_v = wv.rearrange("p (ko j) -> p ko j", ko=KD)
    wo_v = wo.rearrange("p (ko d) -> p ko d", ko=KD)

    from concourse.tile import add_dep_helper

    with tile.TileContext(nc) as tc:
        with (
            tc.tile_pool(name="big", bufs=1) as big_pool,
            tc.tile_pool(name="ag", bufs=3) as ag_pool,
            tc.tile_pool(name="otn", bufs=4) as otn_pool,
            tc.tile_pool(name="wpool", bufs=1) as w_pool,
            tc.tile_pool(name="work", bufs=1) as work_pool,
            tc.tile_pool(name="st", bufs=6) as st_pool,
            tc.tile_pool(name="tree", bufs=10) as tree_pool,
            tc.tile_pool(name="ev", bufs=2) as ev_pool,
            tc.tile_pool(name="psum", bufs=4, space="PSUM") as ps_pool,
            tc.tile_pool(name="psacc", bufs=2, space="PSUM") as psacc_pool,
            tc.tile_pool(name="dram", bufs=1, space="DRAM") as dram_pool,
        ):
            # ---- persistent SBUF tensors ----
            x_sb = big_pool.tile([128, KD, N], bf16, tag="big")
            wq_sb = w_pool.tile([128, KD, JL], bf16, tag="wq")
            wk_sb = w_pool.tile([128, KD, HD], bf16, tag="wk")
            wv_sb = w_pool.tile([128, KD, HD], bf16, tag="wv")
            wo_sb = w_pool.tile([128, KD, DSLICE], bf16, tag="wo")
            cos_sb = w_pool.tile([128, N], bf16, tag="cos")
            qT_sb = work_pool.tile([128, G, N], bf16, tag="qT")
            kT_sb = work_pool.tile([128, N], bf16, tag="kT")
            vT_sb = ag_pool.tile([128, HKV, N], bf16, tag="agsb", name="vT_sb")[:, 0, :]
            v_sb = work_pool.tile([128, N], bf16, tag="v")  # [m-part, mt*128+hd]
            ones_sb = work_pool.tile([128, 128], bf16, tag="ones")
            ident_sb = work_pool.tile([128, 128], bf16, tag="ident")

            nc.gpsimd.memset(ones_sb[:], 1.0)
            make_identity(nc, ident_sb[:])

            # ---- input DMAs (weights needed first come first) ----
            # x rows stay full-width (4KB partition lines — chunking them into
            # 512-col pieces drops DMA throughput to ~160GB/s); the sync queue
            # carries wk + x so nothing slow-lined sits ahead of the x stream.
            nc.sync.dma_start(wk_sb[:], wk_v[:])
            nc.scalar.dma_start(cos_sb[:], cost[:, :])
            nc.scalar.dma_start(wv_sb[:], wv_v[:])
            for kd in range(KD):
                nc.sync.dma_start(x_sb[:, kd, :], xt_v[:, kd, :])
            nc.scalar.dma_start(wq_sb[:], wq_v[:])
            nc.scalar.dma_start(wo_sb[:], wo_v[:])

            # ---- projections ----
            # k first (attention needs full kT before any head starts);
            # kd-outer over 4 live PSUM banks so the PE consumes x rows as
            # they land instead of waiting for the full 8MB
            kps = [
                ps_pool.tile([128, 512], f32, tag="mm", name=f"kps{ncx}")
                for ncx in range(NC512)
            ]
            for kd in range(KD):
                for ncx in range(NC512):
                    nc.tensor.matmul(
                        kps[ncx],
                        lhsT=wk_sb[:, kd, :],
                        rhs=x_sb[:, kd, ts(ncx, 512)],
                        start=(kd == 0),
                        stop=(kd == KD - 1),
                    )
            for ncx in range(NC512):
                nc.vector.tensor_mul(
                    kT_sb[:, ts(ncx, 512)], kps[ncx], cos_sb[:, ts(ncx, 512)]
                )

            # v (as vT, then PE-transpose into natural [m, hd] layout)
            for ncx in range(NC512):
                ps = ps_pool.tile([128, 512], f32, tag="mm")
                for kd in range(KD):
                    nc.tensor.matmul(
                        ps,
                        lhsT=wv_sb[:, kd, :],
                        rhs=x_sb[:, kd, ts(ncx, 512)],
                        start=(kd == 0),
                        stop=(kd == KD - 1),
                    )
                nc.vector.tensor_copy(vT_sb[:, ts(ncx, 512)], ps)

            # q (4 heads); the V PE-transposes (which don't count as HAM
            # activity) are interleaved in small groups so the clock stays warm
            for h in range(G):
                for ncx in range(NC512):
                    ps = ps_pool.tile([128, 512], f32, tag="mm")
                    for kd in range(KD):
                        nc.tensor.matmul(
                            ps,
                            lhsT=wq_sb[:, kd, ts(h, 128)],
                            rhs=x_sb[:, kd, ts(ncx, 512)],
                            start=(kd == 0),
                            stop=(kd == KD - 1),
                        )
                    nc.vector.tensor_mul(
                        qT_sb[:, h, ts(ncx, 512)], ps, cos_sb[:, ts(ncx, 512)]
                    )
                    if h == 0:
                        q4 = ncx
                        ps_t = psacc_pool.tile([128, 512], bf16, tag="sums")
                        for j in range(4):
                            mt = q4 * 4 + j
                            nc.tensor.transpose(
                                ps_t[:, ts(j, 128)], vT_sb[:, ts(mt, 128)], ident_sb[:]
                            )
                        nc.vector.tensor_copy(v_sb[:, ts(q4, 512)], ps_t)

            # outT fp32 accumulator for the output projection (reuses x_sb's slot
            # footprint only after x is dead; separate tag keeps sizes honest)
            outT_acc = big_pool.tile([128, DC, N], f32, tag="big")

            # ---- attention + per-head chunked AllGather + Wo slab accumulation ----
            # heads 0-2: one gather over the full N; head 3 split 3+1 n-chunks so
            # the tail exposes only the small final chunk.
            ag_ins = []
            ag_outs = []
            for h in range(G - 1):
                ag_ins.append(
                    dram_pool.tile([HD, N], bf16, tag=f"agi{h}", name=f"agi{h}")
                )
                ag_outs.append(
                    dram_pool.tile(
                        [HKV * HD, N], bf16, tag=f"ago{h}", name=f"ago{h}"
                    )
                )
            ag_in3a = dram_pool.tile([HD, 1536], bf16, tag="agi3a", name="agi3a")
            ag_out3a = dram_pool.tile(
                [HKV * HD, 1536], bf16, tag="ago3a", name="ago3a"
            )
            ag_in3b = dram_pool.tile([HD, 512], bf16, tag="agi3b", name="agi3b")
            ag_out3b = dram_pool.tile(
                [HKV * HD, 512], bf16, tag="ago3b", name="ago3b"
            )

            # leading tiny collective: absorbs cross-core rendezvous skew while
            # the PE is busy with projections, so the first real gather is cheap
            bar_in = dram_pool.tile([1, 128], bf16, tag="bar_in", name="bar_in")
            bar_out = dram_pool.tile([4, 128], bf16, tag="bar_out", name="bar_out")
            nc.gpsimd.collective_compute(
                "AllGather",
                mybir.AluOpType.bypass,
                replica_groups=[[0, 1, 2, 3], [4, 5, 6, 7]],
                ins=[bar_in[:].opt()],
                outs=[bar_out[:].opt()],
            )

            attn_last = {}

            def attention_chunk(h, ncx):
                """One (head, 512-col n-chunk): scores pipelined one tile ahead
                of exp/AV; softmax sums via DVE pair tree + one ones-matmul."""
                ot_ps = psacc_pool.tile([128, 512], f32, tag="ot")
                s_pend = {}

                def emit_scores(mt):
                    ps = ps_pool.tile([128, 512], f32, tag="mm")
                    nc.tensor.matmul(
                        ps,
                        lhsT=kT_sb[:, ts(mt, 128)],
                        rhs=qT_sb[:, h, ts(ncx, 512)],
                        start=True,
                        stop=True,
                    )
                    s_pend[mt] = ps

                t1 = [None] * 8
                t2 = [None] * 4
                t3 = [None] * 2
                st_tiles = [None] * NT

                emit_scores(0)
                for mt in range(NT):
                    if mt + 1 < NT:
                        emit_scores(mt + 1)
                    st_sb = st_pool.tile([128, 512], bf16, tag="st")
                    nc.scalar.activation(st_sb[:], s_pend.pop(mt), AFT.Exp)
                    st_tiles[mt] = st_sb
                    nc.tensor.matmul(
                        ot_ps,
                        lhsT=v_sb[:, ts(mt, 128)],
                        rhs=st_sb[:],
                        start=(mt == 0),
                        stop=(mt == NT - 1),
                    )
                    # DVE pair-tree for softmax denominators
                    if mt % 2 == 1:
                        j = mt // 2
                        t1[j] = tree_t = tree_pool.tile(
                            [128, 512], bf16, tag="tr", name=f"t1_{h}_{ncx}_{j}"
                        )
                        nc.vector.tensor_add(
                            tree_t[:], st_tiles[mt - 1][:], st_tiles[mt][:]
                        )
                        st_tiles[mt - 1] = st_tiles[mt] = None
                        if j % 2 == 1:
                            j2 = j // 2
                            t2[j2] = tree_t = tree_pool.tile(
                                [128, 512], bf16, tag="tr", name=f"t2_{h}_{ncx}_{j2}"
                            )
                            nc.vector.tensor_add(
                                tree_t[:], t1[j - 1][:], t1[j][:]
                            )
                            t1[j - 1] = t1[j] = None
                            if j2 % 2 == 1:
                                j3 = j2 // 2
                                t3[j3] = tree_t = tree_pool.tile(
                                    [128, 512],
                                    bf16,
                                    tag="tr",
                                    name=f"t3_{h}_{ncx}_{j3}",
                                )
                                nc.vector.tensor_add(
                                    tree_t[:], t2[j2 - 1][:], t2[j2][:]
                                )
                                t2[j2 - 1] = t2[j2] = None
                acc_sb = tree_pool.tile([128, 512], bf16, tag="tr")
                nc.vector.tensor_add(acc_sb[:], t3[0][:], t3[1][:])
                sums_ps = psacc_pool.tile([128, 512], f32, tag="sums")
                sums_mm = nc.tensor.matmul(
                    sums_ps, lhsT=ones_sb[:], rhs=acc_sb[:], start=True, stop=True
                )
                attn_last[h] = sums_mm
                recip_sb = ev_pool.tile([128, 512], f32, tag="recip")
                nc.vector.reciprocal_approx_fast(recip_sb[:], sums_ps)
                otn_sb = otn_pool.tile([128, 512], bf16, tag="otn")
                nc.vector.tensor_mul(otn_sb[:], ot_ps, recip_sb[:])
                return otn_sb

            def slab_contribution(h, ag_sb, nns, gate, out_cols=None):
                """Add gathered head-h slab's term (n-chunks nns) to outT_acc.

                out_cols=(lo, hi): after each dc's adds, fire the out DMA for
                that column range (queues alternate so the tail isn't serial).
                """
                for dc in range(DC):
                    for nn in nns:
                        off = nn - nns[0]
                        ps = ps_pool.tile([128, 512], f32, tag="mm", name=f"sd{h}")
                        for r in range(HKV):
                            jc = r * G + h
                            mm = nc.tensor.matmul(
                                ps,
                                lhsT=wo_sb[:, jc, ts(dc, 128)],
                                rhs=ag_sb[:, r, ts(off, 512)],
                                start=(r == 0),
                                stop=(r == HKV - 1),
                            )
                            if gate is not None:
                                # placement hint: keep slab matmuls out of the
                                # PE stream until the gate head's attention is
                                # done, so the PE never waits on a gather
                                add_dep_helper(
                                    mm.ins,
                                    gate.ins,
                                    sync=True,
                                    reason="slab after later attention",
                                )
                        if h == 0:
                            nc.vector.tensor_copy(outT_acc[:, dc, ts(nn, 512)], ps)
                        else:
                            nc.vector.tensor_add(
                                outT_acc[:, dc, ts(nn, 512)],
                                ps,
                                outT_acc[:, dc, ts(nn, 512)],
                            )
                    if out_cols is not None:
                        lo, hi = out_cols
                        eng = nc.scalar if dc % 2 == 0 else nc.sync
                        eng.dma_start(
                            out[ts(dc, 128), lo:hi], outT_acc[:, dc, lo:hi]
                        )

            def gathered_sb(h, ag_out, ncols, gate, split_r=False):
                ag_v = ag_out.rearrange("(r p) n -> p r n", p=128)
                ag_sb = ag_pool.tile(
                    [128, HKV, ncols], bf16, tag="agsb", name=f"agsb{h}"
                )
                if split_r:
                    for r in range(HKV):
                        eng = nc.sync if r % 2 == 0 else nc.scalar
                        ag_dma = eng.dma_start(ag_sb[:, r, :], ag_v[:, r, :])
                        if gate is not None:
                            add_dep_helper(
                                ag_dma.ins,
                                gate.ins,
                                sync=True,
                                reason="keep slab DMA behind later otn DMAs",
                            )
                else:
                    ag_dma = nc.sync.dma_start(ag_sb[:], ag_v[:])
                    if gate is not None:
                        add_dep_helper(
                            ag_dma.ins,
                            gate.ins,
                            sync=True,
                            reason="keep slab DMA behind later otn DMAs",
                        )
                return ag_sb

            rg = [[0, 1, 2, 3], [4, 5, 6, 7]]
            for h in range(G):
                for ncx in range(NC512):
                    otn_sb = attention_chunk(h, ncx)
                    if h < G - 1:
                        nc.sync.dma_start(ag_ins[h][:, ts(ncx, 512)], otn_sb[:])
                    elif ncx < 3:
                        nc.sync.dma_start(ag_in3a[:, ts(ncx, 512)], otn_sb[:])
                    else:
                        nc.sync.dma_start(ag_in3b[:], otn_sb[:])
                    if h == G - 1 and ncx == 2:
                        nc.gpsimd.collective_compute(
                            "AllGather",
                            mybir.AluOpType.bypass,
                            replica_groups=rg,
                            ins=[ag_in3a[:].opt()],
                            outs=[ag_out3a[:].opt()],
                        )
                if h < G - 1:
                    nc.gpsimd.collective_compute(
                        "AllGather",
                        mybir.AluOpType.bypass,
                        replica_groups=rg,
                        ins=[ag_ins[h][:].opt()],
                        outs=[ag_outs[h][:].opt()],
                    )
                else:
                    nc.gpsimd.collective_compute(
                        "AllGather",
                        mybir.AluOpType.bypass,
                        replica_groups=rg,
                        ins=[ag_in3b[:].opt()],
                        outs=[ag_out3b[:].opt()],
                    )

            # slab matmuls after all attention: gathers 0-2 hide under attention;
            # gather 3a hides under slabs 0-2; only the small 3b chunk is tail
            for h in range(G - 1):
                gate = attn_last[min(h + 1, G - 1)]
                ag_sb = gathered_sb(h, ag_outs[h], N, gate)
                slab_contribution(h, ag_sb, list(range(NC512)), gate)
            gate = attn_last[G - 1]
            ag_sb3a = gathered_sb(3, ag_out3a, 1536, gate, split_r=True)
            ag_sb3b = gathered_sb(3, ag_out3b, 512, gate, split_r=False)
            slab_contribution(3, ag_sb3a, [0, 1, 2], gate, out_cols=(0, 1536))
            slab_contribution(3, ag_sb3b, [3], gate, out_cols=(1536, 2048))

    nc.compile()
    _NC_CACHE["nc"] = nc
    return nc


def kernel(x, Wq, Wk, Wv, Wo):
    _install_axon_ntff_hook()
    import ml_dtypes

    import concourse.bass_utils as bass_utils

    bass_utils.upload_artifacts = lambda tmpdir: str(tmpdir)
    from concourse.bass_utils import run_bass_kernel_spmd

    x = np.asarray(x, dtype=np.float32)
    Wq = np.asarray(Wq, dtype=np.float32)
    Wk = np.asarray(Wk, dtype=np.float32)
    Wv = np.asarray(Wv, dtype=np.float32)
    Wo = np.asarray(Wo, dtype=np.float32)

    bf = ml_dtypes.bfloat16

    def _wlay(w):
        """[D, cols] -> [p=128, ko*cols] matching the (ko p) row split."""
        cols = w.shape[1]
        return np.ascontiguousarray(
            w.reshape(D // 128, 128, cols).transpose(1, 0, 2).reshape(128, -1)
        )

    scale = np.float32(HD**-0.5)
    wq_f = (_fold_rope(Wq, H) * scale).astype(bf)  # [D, 2048]
    wk_f = _fold_rope(Wk, HKV).astype(bf)  # [D, 512]
    wv_f = Wv.astype(bf)  # [D, 512]
    wo_f = Wo.astype(bf)  # [2048, D]
    cos_t = _cos_table().astype(bf)  # [128, N]

    xt = [np.ascontiguousarray(x[b].T).astype(bf) for b in range(B)]

    in_maps = []
    for c in range(N_CORES):
        b, g = divmod(c, HKV)
        in_maps.append(
            {
                "xt": xt[b],
                "wq": _wlay(wq_f[:, g * JL : (g + 1) * JL]),
                "wk": _wlay(wk_f[:, g * HD : (g + 1) * HD]),
                "wv": _wlay(wv_f[:, g * HD : (g + 1) * HD]),
                "wo": _wlay(wo_f[:, g * DSLICE : (g + 1) * DSLICE]),
                "cost": cos_t,
            }
        )

    nc = _build_nc()
    res = run_bass_kernel_spmd(nc, in_maps, list(range(N_CORES)))

    out = np.empty((B, N, D), dtype=np.float32)
    for c in range(N_CORES):
        b, g = divmod(c, HKV)
        out[b, :, g * DSLICE : (g + 1) * DSLICE] = res.results[c]["out"].T
    return out


# revision 10
# speedup vs baseline: 1.0749x; 1.0151x over previous
"""GroupedQueryAttention (B=2, N=2048, D=2048, H=16, HKV=4, HD=128) on 8 trn2 cores.

Sharding: core c handles (batch b = c//4, kv-head g = c%4): 4 q-heads + 1 kv head.
RoPE (with the reference's sin==cos quirk) is folded into Wq/Wk host-side:
  q_rot = cos ⊙ (M q) with constant M = [[I,-I],[I,I]] acting on head-dim halves,
so on-device RoPE is just an elementwise multiply by a precomputed cos table.
The softmax scale is folded into Wq. All matmuls run in bf16 with fp32 PSUM
accumulation. Attention uses a transpose-free layout chain:
  qT,kT [hd,n]; ST = kT.T @ qT [m,n]; exp on ScalarE; OT += v.T@ST (v as [m,hd]).
The scores loop is software-pipelined one tile ahead so the PE never stalls on
the ScalarE exp. Softmax denominators: DVE pairwise-add tree over the 16 exp
tiles, then a single all-ones stationary matmul per (head, n-chunk); normalize
with DVE fast-reciprocal. Per-head chunked AllGathers (bf16) overlap with
attention of later heads (head 3's gather is split 3+1 n-chunks to shrink the
tail); the output projection accumulates per gathered slab into an SBUF fp32
accumulator in transposed [d, n] layout (Wo chunks stationary), and the host
transposes each core's [512, 2048] slice back.
Host gathers: out[b][:, g*512:(g+1)*512] = core (b,g) output transposed.
"""

import sys
import types

import numpy as np

B, N, D = 2, 2048, 2048
H, HKV, HD = 16, 4, 128
G = H // HKV  # q heads per kv head = 4
N_CORES = 8
ROPE_BASE = 10000.0
DSLICE = D // G  # 512 output columns per core
JL = G * HD  # 512 local attention-output rows per core


def _install_axon_ntff_hook():
    """This container's antenv lacks axon_hooks; inject it so trace=True works."""
    if "antenv.axon_hooks" in sys.modules:
        return
    try:
        from trn_agent_boot.trn_boot import _ntff_profile_via_ctypes

        hook = _ntff_profile_via_ctypes("/opt/axon/libaxon_pjrt.so")
    except Exception:
        hook = None
    mod = types.ModuleType("antenv.axon_hooks")
    mod.get_axon_ntff_profile_hook = lambda: hook
    mod.set_axon_ntff_profile_hook = lambda h: None
    sys.modules["antenv.axon_hooks"] = mod


def _fold_rope(w: np.ndarray, n_heads: int) -> np.ndarray:
    """Return W' with the (sin==cos) RoPE mixing folded in: x@W' = M(x@W) per head."""
    wf = w.reshape(D, n_heads, HD)
    lo, hi = wf[..., : HD // 2], wf[..., HD // 2 :]
    return np.concatenate([lo - hi, hi + lo], axis=-1).reshape(D, n_heads * HD)


def _cos_table() -> np.ndarray:
    inv_freq = 1.0 / (ROPE_BASE ** (np.arange(0, HD, 2, dtype=np.float64) / HD))
    freqs = np.arange(N, dtype=np.float64)[:, None] * inv_freq[None, :]  # [N, 64]
    emb = np.concatenate([freqs, freqs], axis=-1)  # [N, 128]
    return np.cos(emb).T.astype(np.float32).copy()  # [128, N]


_NC_CACHE: dict = {}


def _build_nc():
    if "nc" in _NC_CACHE:
        return _NC_CACHE["nc"]

    import concourse.bacc as bacc
    import concourse.mybir as mybir
    import concourse.tile as tile
    from concourse.bass import ts
    from concourse.masks import make_identity

    f32 = mybir.dt.float32
    bf16 = mybir.dt.bfloat16
    AFT = mybir.ActivationFunctionType
    KD = D // 128  # 16 contraction chunks
    NT = N // 128  # 16 m tiles of 128
    NC512 = N // 512  # 4 chunks of 512
    DC = DSLICE // 128  # 4 output-column chunks of 128

    nc = bacc.Bacc(target_bir_lowering=False, debug=False, num_devices=N_CORES)

    xt = nc.dram_tensor("xt", [D, N], bf16, kind="ExternalInput")  # x[b].T
    # weights arrive pre-rearranged host-side to [p, ko, cols] so their DMAs
    # have long contiguous partition lines (the (ko p) row-interleaved view
    # yields 256B lines and ~5x slower DMA)
    wq = nc.dram_tensor("wq", [128, KD * JL], bf16, kind="ExternalInput")
    wk = nc.dram_tensor("wk", [128, KD * HD], bf16, kind="ExternalInput")
    wv = nc.dram_tensor("wv", [128, KD * HD], bf16, kind="ExternalInput")
    wo = nc.dram_tensor("wo", [128, KD * DSLICE], bf16, kind="ExternalInput")
    cost = nc.dram_tensor("cost", [HD, N], bf16, kind="ExternalInput")
    # transposed output: outT[d, n]; host transposes back
    out = nc.dram_tensor("out", [DSLICE, N], f32, kind="ExternalOutput")

    xt_v = xt.rearrange("(ko p) n -> p ko n", p=128)
    wq_v = wq.rearrange("p (ko j) -> p ko j", ko=KD)
    wk_v = wk.rearrange("p (ko j) -> p ko j", ko=KD)
    wv_v = wv.rearrange("p (ko j) -> p ko j", ko=KD)
    wo_v = wo.rearrange("p (ko d) -> p ko d", ko=KD)

    from concourse.tile import add_dep_helper

    with tile.TileContext(nc) as tc:
        with (
            tc.tile_pool(name="big", bufs=1) as big_pool,
            tc.tile_pool(name="ag", bufs=3) as ag_pool,
            tc.tile_pool(name="otn", bufs=4) as otn_pool,
            tc.tile_pool(name="wpool", bufs=1) as w_pool,
            tc.tile_pool(name="work", bufs=1) as work_pool,
            tc.tile_pool(name="st", bufs=6) as st_pool,
            tc.tile_pool(name="tree", bufs=10) as tree_pool,
            tc.tile_pool(name="ev", bufs=2) as ev_pool,
            tc.tile_pool(name="psum", bufs=4, space="PSUM") as ps_pool,
            tc.tile_pool(name="psacc", bufs=2, space="PSUM") as psacc_pool,
            tc.tile_pool(name="dram", bufs=1, space="DRAM") as dram_pool,
        ):
            # ---- persistent SBUF tensors ----
            x_sb = big_pool.tile([128, KD, N], bf16, tag="big")
            wq_sb = w_pool.tile([128, KD, JL], bf16, tag="wq")
            wk_sb = w_pool.tile([128, KD, HD], bf16, tag="wk")
            wv_sb = w_pool.tile([128, KD, HD], bf16, tag="wv")
            wo_sb = w_pool.tile([128, KD, DSLICE], bf16, tag="wo")
            cos_sb = w_pool.tile([128, N], bf16, tag="cos")
            qT_sb = work_pool.tile([128, G, N], bf16, tag="qT")
            kT_sb = work_pool.tile([128, N], bf16, tag="kT")
            vT_sb = ag_pool.tile([128, HKV, N], bf16, tag="agsb", name="vT_sb")[:, 0, :]
            v_sb = work_pool.tile([128, N], bf16, tag="v")  # [m-part, mt*128+hd]
            ones_sb = work_pool.tile([128, 128], bf16, tag="ones")
            ident_sb = work_pool.tile([128, 128], bf16, tag="ident")
            warm_sb = work_pool.tile([128, 512], bf16, tag="warm")

            nc.gpsimd.memset(ones_sb[:], 1.0)
            make_identity(nc, ident_sb[:])
            nc.gpsimd.memset(warm_sb[:], 0.5)

            # ---- input DMAs (weights needed first come first) ----
            # x rows stay full-width (4KB partition lines — chunking them into
            # 512-col pieces drops DMA throughput to ~160GB/s); the sync queue
            # carries wk + x so nothing slow-lined sits ahead of the x stream.
            nc.sync.dma_start(wk_sb[:], wk_v[:])
            nc.scalar.dma_start(cos_sb[:], cost[:, :])
            nc.scalar.dma_start(wv_sb[:], wv_v[:])
            for kd in range(KD):
                nc.sync.dma_start(x_sb[:, kd, 0:1024], xt_v[:, kd, 0:1024])
                nc.scalar.dma_start(x_sb[:, kd, 1024:2048], xt_v[:, kd, 1024:2048])
            nc.sync.dma_start(wq_sb[:], wq_v[:])
            nc.scalar.dma_start(wo_sb[:], wo_v[:])

            # ---- projections ----
            # k first (attention needs full kT before any head starts);
            # kd-outer over 4 live PSUM banks so the PE consumes x rows as
            # they land instead of waiting for the full 8MB
            # HAM warm-up: the K proj below is DMA-paced, and its idle gaps
            # would otherwise hold the PE clock at 1.2GHz for the first ~35us.
            # Dummy matmuls (no data deps) warm the clock and keep-warm fillers
            # bridge the x-row waits.
            warm_ps = psacc_pool.tile([128, 512], f32, tag="ot", name="warm_ps")

            def warm_mms(k):
                for _ in range(k):
                    nc.tensor.matmul(
                        warm_ps, lhsT=ones_sb[:], rhs=warm_sb[:], start=True,
                        stop=True,
                    )

            warm_mms(20)
            kps = [
                ps_pool.tile([128, 512], f32, tag="mm", name=f"kps{ncx}")
                for ncx in range(NC512)
            ]
            for kd in range(KD):
                for ncx in range(NC512):
                    nc.tensor.matmul(
                        kps[ncx],
                        lhsT=wk_sb[:, kd, :],
                        rhs=x_sb[:, kd, ts(ncx, 512)],
                        start=(kd == 0),
                        stop=(kd == KD - 1),
                    )
                if kd < 12:
                    warm_mms(3)
            for ncx in range(NC512):
                nc.vector.tensor_mul(
                    kT_sb[:, ts(ncx, 512)], kps[ncx], cos_sb[:, ts(ncx, 512)]
                )

            # v (as vT, then PE-transpose into natural [m, hd] layout)
            for ncx in range(NC512):
                ps = ps_pool.tile([128, 512], f32, tag="mm")
                for kd in range(KD):
                    nc.tensor.matmul(
                        ps,
                        lhsT=wv_sb[:, kd, :],
                        rhs=x_sb[:, kd, ts(ncx, 512)],
                        start=(kd == 0),
                        stop=(kd == KD - 1),
                    )
                nc.vector.tensor_copy(vT_sb[:, ts(ncx, 512)], ps)

            # q (4 heads); the V PE-transposes (which don't count as HAM
            # activity) are interleaved in small groups so the clock stays warm
            for h in range(G):
                for ncx in range(NC512):
                    ps = ps_pool.tile([128, 512], f32, tag="mm")
                    for kd in range(KD):
                        nc.tensor.matmul(
                            ps,
                            lhsT=wq_sb[:, kd, ts(h, 128)],
                            rhs=x_sb[:, kd, ts(ncx, 512)],
                            start=(kd == 0),
                            stop=(kd == KD - 1),
                        )
                    nc.vector.tensor_mul(
                        qT_sb[:, h, ts(ncx, 512)], ps, cos_sb[:, ts(ncx, 512)]
                    )
                    if h == 0:
                        q4 = ncx
                        ps_t = psacc_pool.tile([128, 512], bf16, tag="sums")
                        for j in range(4):
                            mt = q4 * 4 + j
                            nc.tensor.transpose(
                                ps_t[:, ts(j, 128)], vT_sb[:, ts(mt, 128)], ident_sb[:]
                            )
                        nc.vector.tensor_copy(v_sb[:, ts(q4, 512)], ps_t)

            # outT fp32 accumulator for the output projection (reuses x_sb's slot
            # footprint only after x is dead; separate tag keeps sizes honest)
            outT_acc = big_pool.tile([128, DC, N], f32, tag="big")

            # ---- attention + per-head chunked AllGather + Wo slab accumulation ----
            # heads 0-2: one gather over the full N; head 3 split 3+1 n-chunks so
            # the tail exposes only the small final chunk.
            ag_ins = []
            ag_outs = []
            for h in range(G - 1):
                ag_ins.append(
                    dram_pool.tile([HD, N], bf16, tag=f"agi{h}", name=f"agi{h}")
                )
                ag_outs.append(
                    dram_pool.tile(
                        [HKV * HD, N], bf16, tag=f"ago{h}", name=f"ago{h}"
                    )
                )
            ag_in3a = dram_pool.tile([HD, 1536], bf16, tag="agi3a", name="agi3a")
            ag_out3a = dram_pool.tile(
                [HKV * HD, 1536], bf16, tag="ago3a", name="ago3a"
            )
            ag_in3b = dram_pool.tile([HD, 512], bf16, tag="agi3b", name="agi3b")
            ag_out3b = dram_pool.tile(
                [HKV * HD, 512], bf16, tag="ago3b", name="ago3b"
            )

            # leading tiny collective: absorbs cross-core rendezvous skew while
            # the PE is busy with projections, so the first real gather is cheap
            bar_in = dram_pool.tile([1, 128], bf16, tag="bar_in", name="bar_in")
            bar_out = dram_pool.tile([4, 128], bf16, tag="bar_out", name="bar_out")
            nc.gpsimd.collective_compute(
                "AllGather",
                mybir.AluOpType.bypass,
                replica_groups=[[0, 1, 2, 3], [4, 5, 6, 7]],
                ins=[bar_in[:].opt()],
                outs=[bar_out[:].opt()],
            )

            attn_last = {}

            def attention_chunk(h, ncx):
                """One (head, 512-col n-chunk): scores pipelined one tile ahead
                of exp/AV; softmax sums via DVE pair tree + one ones-matmul."""
                ot_ps = psacc_pool.tile([128, 512], f32, tag="ot")
                s_pend = {}

                def emit_scores(mt):
                    ps = ps_pool.tile([128, 512], f32, tag="mm")
                    nc.tensor.matmul(
                        ps,
                        lhsT=kT_sb[:, ts(mt, 128)],
                        rhs=qT_sb[:, h, ts(ncx, 512)],
                        start=True,
                        stop=True,
                    )
                    s_pend[mt] = ps

                t1 = [None] * 8
                t2 = [None] * 4
                t3 = [None] * 2
                st_tiles = [None] * NT

                emit_scores(0)
                for mt in range(NT):
                    if mt + 1 < NT:
                        emit_scores(mt + 1)
                    st_sb = st_pool.tile([128, 512], bf16, tag="st")
                    nc.scalar.activation(st_sb[:], s_pend.pop(mt), AFT.Exp)
                    st_tiles[mt] = st_sb
                    nc.tensor.matmul(
                        ot_ps,
                        lhsT=v_sb[:, ts(mt, 128)],
                        rhs=st_sb[:],
                        start=(mt == 0),
                        stop=(mt == NT - 1),
                    )
                    # DVE pair-tree for softmax denominators
                    if mt % 2 == 1:
                        j = mt // 2
                        t1[j] = tree_t = tree_pool.tile(
                            [128, 512], bf16, tag="tr", name=f"t1_{h}_{ncx}_{j}"
                        )
                        nc.vector.tensor_add(
                            tree_t[:], st_tiles[mt - 1][:], st_tiles[mt][:]
                        )
                        st_tiles[mt - 1] = st_tiles[mt] = None
                        if j % 2 == 1:
                            j2 = j // 2
                            t2[j2] = tree_t = tree_pool.tile(
                                [128, 512], bf16, tag="tr", name=f"t2_{h}_{ncx}_{j2}"
                            )
                            nc.vector.tensor_add(
                                tree_t[:], t1[j - 1][:], t1[j][:]
                            )
                            t1[j - 1] = t1[j] = None
                            if j2 % 2 == 1:
                                j3 = j2 // 2
                                t3[j3] = tree_t = tree_pool.tile(
                                    [128, 512],
                                    bf16,
                                    tag="tr",
                                    name=f"t3_{h}_{ncx}_{j3}",
                                )
                                nc.vector.tensor_add(
                                    tree_t[:], t2[j2 - 1][:], t2[j2][:]
                                )
                                t2[j2 - 1] = t2[j2] = None
                acc_sb = tree_pool.tile([128, 512], bf16, tag="tr")
                nc.vector.tensor_add(acc_sb[:], t3[0][:], t3[1][:])
                sums_ps = psacc_pool.tile([128, 512], f32, tag="sums")
                sums_mm = nc.tensor.matmul(
                    sums_ps, lhsT=ones_sb[:], rhs=acc_sb[:], start=True, stop=True
                )
                attn_last[h] = sums_mm
                recip_sb = ev_pool.tile([128, 512], f32, tag="recip")
                nc.vector.reciprocal_approx_fast(recip_sb[:], sums_ps)
                otn_sb = otn_pool.tile([128, 512], bf16, tag="otn")
                nc.vector.tensor_mul(otn_sb[:], ot_ps, recip_sb[:])
                return otn_sb

            def slab_contribution(h, ag_sb, nns, gate, out_cols=None):
                """Add gathered head-h slab's term (n-chunks nns) to outT_acc.

                out_cols=(lo, hi): after each dc's adds, fire the out DMA for
                that column range (queues alternate so the tail isn't serial).
                """
                for dc in range(DC):
                    for nn in nns:
                        off = nn - nns[0]
                        ps = ps_pool.tile([128, 512], f32, tag="mm", name=f"sd{h}")
                        for r in range(HKV):
                            jc = r * G + h
                            mm = nc.tensor.matmul(
                                ps,
                                lhsT=wo_sb[:, jc, ts(dc, 128)],
                                rhs=ag_sb[:, r, ts(off, 512)],
                                start=(r == 0),
                                stop=(r == HKV - 1),
                            )
                            if gate is not None:
                                # placement hint: keep slab matmuls out of the
                                # PE stream until the gate head's attention is
                                # done, so the PE never waits on a gather
                                add_dep_helper(
                                    mm.ins,
                                    gate.ins,
                                    sync=True,
                                    reason="slab after later attention",
                                )
                        if h == 0:
                            nc.vector.tensor_copy(outT_acc[:, dc, ts(nn, 512)], ps)
                        else:
                            nc.vector.tensor_add(
                                outT_acc[:, dc, ts(nn, 512)],
                                ps,
                                outT_acc[:, dc, ts(nn, 512)],
                            )
                    if out_cols is not None:
                        lo, hi = out_cols
                        eng = nc.scalar if dc % 2 == 0 else nc.sync
                        eng.dma_start(
                            out[ts(dc, 128), lo:hi], outT_acc[:, dc, lo:hi]
                        )

            def gathered_sb(h, ag_out, ncols, gate, split_r=False):
                ag_v = ag_out.rearrange("(r p) n -> p r n", p=128)
                ag_sb = ag_pool.tile(
                    [128, HKV, ncols], bf16, tag="agsb", name=f"agsb{h}"
                )
                if split_r:
                    for r in range(HKV):
                        eng = nc.sync if r % 2 == 0 else nc.scalar
                        ag_dma = eng.dma_start(ag_sb[:, r, :], ag_v[:, r, :])
                        if gate is not None:
                            add_dep_helper(
                                ag_dma.ins,
                                gate.ins,
                                sync=True,
                                reason="keep slab DMA behind later otn DMAs",
                            )
                else:
                    ag_dma = nc.sync.dma_start(ag_sb[:], ag_v[:])
                    if gate is not None:
                        add_dep_helper(
                            ag_dma.ins,
                            gate.ins,
                            sync=True,
                            reason="keep slab DMA behind later otn DMAs",
                        )
                return ag_sb

            rg = [[0, 1, 2, 3], [4, 5, 6, 7]]
            for h in range(G):
                for ncx in range(NC512):
                    otn_sb = attention_chunk(h, ncx)
                    if h < G - 1:
                        nc.sync.dma_start(ag_ins[h][:, ts(ncx, 512)], otn_sb[:])
                    elif ncx < 3:
                        nc.sync.dma_start(ag_in3a[:, ts(ncx, 512)], otn_sb[:])
                    else:
                        nc.sync.dma_start(ag_in3b[:], otn_sb[:])
                    if h == G - 1 and ncx == 2:
                        nc.gpsimd.collective_compute(
                            "AllGather",
                            mybir.AluOpType.bypass,
                            replica_groups=rg,
                            ins=[ag_in3a[:].opt()],
                            outs=[ag_out3a[:].opt()],
                        )
                if h < G - 1:
                    nc.gpsimd.collective_compute(
                        "AllGather",
                        mybir.AluOpType.bypass,
                        replica_groups=rg,
                        ins=[ag_ins[h][:].opt()],
                        outs=[ag_outs[h][:].opt()],
                    )
                else:
                    nc.gpsimd.collective_compute(
                        "AllGather",
                        mybir.AluOpType.bypass,
                        replica_groups=rg,
                        ins=[ag_in3b[:].opt()],
                        outs=[ag_out3b[:].opt()],
                    )

            # slab matmuls after all attention: gathers 0-2 hide under attention;
            # gather 3a hides under slabs 0-2; only the small 3b chunk is tail
            for h in range(G - 1):
                gate = attn_last[min(h + 1, G - 1)]
                ag_sb = gathered_sb(h, ag_outs[h], N, gate)
                slab_contribution(h, ag_sb, list(range(NC512)), gate)
            gate = attn_last[G - 1]
            ag_sb3a = gathered_sb(3, ag_out3a, 1536, gate, split_r=True)
            ag_sb3b = gathered_sb(3, ag_out3b, 512, gate, split_r=False)
            slab_contribution(3, ag_sb3a, [0, 1, 2], gate, out_cols=(0, 1536))
            slab_contribution(3, ag_sb3b, [3], gate, out_cols=(1536, 2048))

    nc.compile()
    _NC_CACHE["nc"] = nc
    return nc


def kernel(x, Wq, Wk, Wv, Wo):
    _install_axon_ntff_hook()
    import ml_dtypes

    import concourse.bass_utils as bass_utils

    bass_utils.upload_artifacts = lambda tmpdir: str(tmpdir)
    from concourse.bass_utils import run_bass_kernel_spmd

    x = np.asarray(x, dtype=np.float32)
    Wq = np.asarray(Wq, dtype=np.float32)
    Wk = np.asarray(Wk, dtype=np.float32)
    Wv = np.asarray(Wv, dtype=np.float32)
    Wo = np.asarray(Wo, dtype=np.float32)

    bf = ml_dtypes.bfloat16

    def _wlay(w):
        """[D, cols] -> [p=128, ko*cols] matching the (ko p) row split."""
        cols = w.shape[1]
        return np.ascontiguousarray(
            w.reshape(D // 128, 128, cols).transpose(1, 0, 2).reshape(128, -1)
        )

    scale = np.float32(HD**-0.5)
    wq_f = (_fold_rope(Wq, H) * scale).astype(bf)  # [D, 2048]
    wk_f = _fold_rope(Wk, HKV).astype(bf)  # [D, 512]
    wv_f = Wv.astype(bf)  # [D, 512]
    wo_f = Wo.astype(bf)  # [2048, D]
    cos_t = _cos_table().astype(bf)  # [128, N]

    xt = [np.ascontiguousarray(x[b].T).astype(bf) for b in range(B)]

    in_maps = []
    for c in range(N_CORES):
        b, g = divmod(c, HKV)
        in_maps.append(
            {
                "xt": xt[b],
                "wq": _wlay(wq_f[:, g * JL : (g + 1) * JL]),
                "wk": _wlay(wk_f[:, g * HD : (g + 1) * HD]),
                "wv": _wlay(wv_f[:, g * HD : (g + 1) * HD]),
                "wo": _wlay(wo_f[:, g * DSLICE : (g + 1) * DSLICE]),
                "cost": cos_t,
            }
        )

    nc = _build_nc()
    res = run_bass_kernel_spmd(nc, in_maps, list(range(N_CORES)))

    out = np.empty((B, N, D), dtype=np.float32)
    for c in range(N_CORES):
        b, g = divmod(c, HKV)
        out[b, :, g * DSLICE : (g + 1) * DSLICE] = res.results[c]["out"].T
    return out
